# revision 21
# baseline (speedup 1.0000x reference)
"""Trainium2 Bass kernel for nn_DivEncLayer (grouped per-slice MLP 8->32->1).

Reference computation (per batch row b, per slice q of 128):
    xs = x.reshape(B, 128, 8)
    h  = ELU(xs[b,q,:] @ W1[q] + b1[q])            # (32,)
    h  = (h - mov_mean[q]) * gamma[q]/sqrt(mov_var[q]+eps) + beta[q]
    out[b,q] = h @ W2[q] + b2[q]

Strategy (pure data parallel over 8 NeuronCores, B=32768 -> 4096/core):
  * BN affine + W2 fold into w2p[q,h] (host); final bias bfin[q] (host).
  * ELU identity used on-chip:  elu(u) = max(u, min(e^u, 1) - 1).
    With s = 65504 and u65 := s*u produced directly by dense1 (weights
    pre-scaled by s on host), per u-tile we run exactly three passes:
      ACT :  E   = Exp(u65/s + ln s)          = s*e^u        (bf16)
      DVE/Pool: T = (E min s) add -s          = s*(min(e^u,1)-1)
      DVE :  C   = (u65 add s*b1) max T       = s*elu(u+b1)  (bf16)
    C feeds a SINGLE dense2 stream with lhsT = w2p/s.
  * x is pre-scaled by 8 and converted to fp16 on host (w1 carries s/8),
    then loaded TRANSPOSED straight from DRAM into SBUF via the DMA xbar
    transpose (InstDmaTransposeAnt) - no PE transposes / PSUM drains on
    the input path, and half the HBM traffic of f32.
  * dense1: per c-group of 16 slices, 4 block-diagonal K=128 matmuls
    (fp16) into 2 PSUM tiles [128, 2, 512]; partition p = 32j + h.
  * dense2: per pair of groups, 8 matmuls (bf16, K=128) accumulate
    out[q, b] into one PSUM bank; partition index == q.
  * Output: single ACT bias-add pass [128q, 512b], PE transpose back to
    [b, q] (f32), ACT drain, DMA out.

Known walrus/HW constraints handled here:
  * any instruction encoding supports only ONE semaphore wait -> _split_waits
  * PSUM accumulation chains must share one tile_position
  * matmul PSUM output base partition must be 32-aligned
  * float->float downcast on TRN does NOT saturate (gives inf), so all
    scaled intermediates are kept in bf16 (wide exponent range).
"""

import sys

for _p in ("/opt/trn_rl_repo", "/root/.axon_site/_ro/trn_rl_repo"):
    if _p not in sys.path:
        sys.path.append(_p)

import contextlib
import math
import os as _os

import numpy as np

import concourse.bass as bass
import concourse.tile as tile
from concourse import mybir
from concourse.bass_utils import run_bass_kernel_spmd
from concourse.masks import make_identity

F32 = mybir.dt.float32
F32R = mybir.dt.float32r
BF16 = mybir.dt.bfloat16
F16 = mybir.dt.float16

Q, S, H = 128, 8, 32
C = Q * S                      # 1024
NCORES = 8
BN_EPS = 1e-3

NB = 512                       # batch tile (matmul free dim)
NG = 8                         # c/slice groups of 16 slices (128 partitions)

SCALE = 65504.0                # s: act streams carry s*elu(u)
XS = 8.0                       # host pre-scale on x (fp16 range headroom)
LN_S = math.log(SCALE)

# groups whose min-pass runs on GpSimd (rest on DVE); DVE also runs all stt
MIN_POOL_GROUPS = int(_os.environ.get("MIN_POOL_GROUPS", "8"))
# (group, half) units that skip the stt and instead use two dense2 streams
# (R = ReLU on ACT, T = the min output): offloads DVE onto ACT + PE.
# RT_HALVES = k converts the LAST k of the 32 quarter-units per tile.
RT_HALVES = int(_os.environ.get("RT_HALVES", "1"))

_NOPN = [0]


def _split_waits(tc):
    """walrus supports only one sync-wait command per instruction; Tile can
    emit several.  Precede every multi-wait instruction with same-engine
    NoOps carrying all but the last wait."""
    orig = tc._add_instruction

    def patched(inst):
        si = inst.sync_info
        if (
            not inst.name.startswith("waitnop")
            and si is not None
            and len(si.on_wait) > 1
        ):
            for w in si.on_wait[:-1]:
                _NOPN[0] += 1
                nop = mybir.InstNoOp(name=f"waitnop-{_NOPN[0]}", ins=[], outs=[])
                nop.engine = inst.engine
                nop.sync_info = mybir.SyncInfo(on_wait=[w], on_update=[])
                orig(nop)
            inst.sync_info = mybir.SyncInfo(
                on_wait=[si.on_wait[-1]], on_update=list(si.on_update)
            )
        return orig(inst)

    tc._add_instruction = patched

    def patched_dab(tick_clock, wait_clock):
        from concourse.vector_clock import ScopedClock

        nc = tc.nc
        drain_inst = nc.sync.drain()
        wait_clock.add_sem_waits(
            drain_inst.ins, ScopedClock({None: tick_clock.global_clock})
        )
        si = drain_inst.ins.sync_info
        if si is not None and len(si.on_wait) > 1:
            extra = list(si.on_wait[1:])
            drain_inst.ins.sync_info = mybir.SyncInfo(
                on_wait=[si.on_wait[0]], on_update=list(si.on_update)
            )
            for w in extra:
                n = nc.sync.nop(nofuse=True)
                n.ins.sync_info = mybir.SyncInfo(on_wait=[w], on_update=[])

        nc.all_engine_barrier()
        assert tc.sems is not None
        popped = nc._tile_sem_poison_stack.pop()
        assert popped is tc._sem_poison
        nc.clear_and_free_semaphores(list(tc.sems.allocated().values()))
        nc.all_engine_barrier()

    tc._drain_and_barrier = patched_dab


def _host_pack(W1, b1, gamma, beta, mov_mean, mov_var, W2, b2):
    """Fold BN into second dense; pack block weights for the PE layouts."""
    import ml_dtypes

    W1 = np.asarray(W1, np.float32).reshape(Q, S, H)
    b1 = np.asarray(b1, np.float32).reshape(Q, H)
    gamma = np.asarray(gamma, np.float32).reshape(Q, H)
    beta = np.asarray(beta, np.float32).reshape(Q, H)
    mean = np.asarray(mov_mean, np.float32).reshape(Q, H)
    var = np.asarray(mov_var, np.float32).reshape(Q, H)
    W2 = np.asarray(W2, np.float32).reshape(Q, H)
    b2 = np.asarray(b2, np.float32).reshape(Q)

    inv = gamma / np.sqrt(var + BN_EPS)
    w2p = (inv * W2).astype(np.float32)                      # [Q,H]
    # out = sum_h (w2p/s) * (s*elu(u)) + bfin
    bfin = (b2 + ((beta - mean * inv) * W2).sum(-1)).astype(np.float32)

    # dense1 stationary blocks: MM (g,i) is a K=128 matmul with a
    # block-diagonal lhsT (rows 32i..32i+32 live) computing slices
    # q=16g+4i+j at output partitions 32j+h.  Weights carry SCALE/XS so
    # that PSUM = SCALE * u with x pre-scaled by XS.
    w1bd = np.zeros((128, NG, 4, 128), np.float32)
    for g in range(NG):
        for i in range(4):
            for j in range(4):
                q = 16 * g + 4 * i + j
                w1bd[32 * i + 8 * j:32 * i + 8 * j + 8, g, i, 32 * j:32 * j + 32] = (
                    W1[q] * (SCALE / XS)
                )
    w1bd = w1bd.astype(np.float16)

    # dense2 block-diagonal lhsT: col m holds w2p/s of slice q=16g+4i+j at
    # rows 32j..32j+32, with m = 16*(g%2)+4i+j so output partition == q.
    w2t = np.zeros((128, NG, 4, 32), np.float32)
    for g in range(NG):
        for i in range(4):
            for j in range(4):
                q = 16 * g + 4 * i + j
                m = 16 * (g % 2) + 4 * i + j
                w2t[32 * j:32 * j + 32, g, i, m] = w2p[q] / SCALE
    w2t = w2t.astype(ml_dtypes.bfloat16)

    # per-partition s*b1 for the (rare) b1 != 0 path: [p=32j+h, g, i]
    b1sb = np.zeros((128, NG, 4, 1), np.float32)
    for g in range(NG):
        for i in range(4):
            for j in range(4):
                q = 16 * g + 4 * i + j
                b1sb[32 * j:32 * j + 32, g, i, 0] = b1[q] * SCALE
    has_b1 = bool(np.any(b1 != 0.0))
    # Exp bias: ln(s) + b1 per partition (collapses to ln(s) when b1 == 0)
    b1e = (LN_S + b1sb / SCALE).astype(np.float32)

    return w1bd, w2t, bfin.reshape(128, 1), b1sb, b1e, has_b1


def _build(bc, has_b1, rep=1):
    """Build the Bass program for one core processing bc batch rows.

    rep>1 wraps the batch loop in a For loop reprocessing the same data
    (benchmarking only: amplifies kernel time over ~90ms axon dispatch)."""
    nc = bass.Bass()

    x_d = nc.dram_tensor("x", [bc, C], F16, kind="ExternalInput")
    w1_d = nc.dram_tensor("w1bd", [128, NG, 4, 128], F16, kind="ExternalInput")
    w2_d = nc.dram_tensor("w2t", [128, NG, 4, 32], BF16, kind="ExternalInput")
    bf_d = nc.dram_tensor("bfin", [128, 1], F32, kind="ExternalInput")
    b1_d = nc.dram_tensor("b1sb", [128, NG, 4, 1], F32, kind="ExternalInput")
    b1e_d = nc.dram_tensor("b1e", [128, NG, 4, 1], F32, kind="ExternalInput")
    out_d = nc.dram_tensor("out", [bc, 128], F16, kind="ExternalOutput")

    n_tiles = bc // NB
    Exp = mybir.ActivationFunctionType.Exp
    Relu = mybir.ActivationFunctionType.Relu
    Identity = mybir.ActivationFunctionType.Identity
    Copy = mybir.ActivationFunctionType.Copy
    Alu = mybir.AluOpType

    with tile.TileContext(nc) as tc:
        _split_waits(tc)
        with (
            tc.tile_pool(name="singles", bufs=1) as singles,
            tc.tile_pool(name="xt", bufs=12) as xt_pool,
            tc.tile_pool(name="ew", bufs=4) as ew_pool,
            tc.tile_pool(name="tm", bufs=4) as tm_pool,
            tc.tile_pool(name="cw", bufs=8) as cw_pool,
            tc.tile_pool(name="outq", bufs=2) as outq_pool,
            tc.tile_pool(name="outb", bufs=2) as outb_pool,
            tc.tile_pool(name="ps_u", bufs=6, space="PSUM") as ps_u,
            tc.tile_pool(name="ps_o", bufs=2, space="PSUM") as ps_o,
        ):
            w1t = singles.tile([128, NG, 4, 128], F16)
            w2t = singles.tile([128, NG, 4, 32], BF16)
            bfin = singles.tile([128, 1], F32)
            b1sb = singles.tile([128, NG, 4, 1], F32)
            b1e = singles.tile([128, NG, 4, 1], F32)
            identh = singles.tile([128, 128], F16)
            identb = singles.tile([128, 128], BF16)
            wdum = singles.tile([128, 8], F32)

            nc.sync.dma_start(w1t[:], w1_d[:])
            nc.sync.dma_start(w2t[:], w2_d[:])
            nc.sync.dma_start(bfin[:], bf_d[:])
            nc.sync.dma_start(b1sb[:], b1_d[:])
            nc.sync.dma_start(b1e[:], b1e_d[:])
            make_identity(nc, identh[:])
            make_identity(nc, identb[:])

            # Warmup: make each engine observe each one-time producer once so
            # steady-state instructions need at most one semaphore wait.
            pdum = ps_o.tile([128, NB], F32, tag="o", name="pdum")
            nc.tensor.transpose(pdum[0:1, 0:64].bitcast(F16), w1t[:, 0, 0, 0:1], identh[:])
            nc.tensor.transpose(pdum[0:1, 64:128].bitcast(BF16), w2t[:, 0, 0, 0:1], identb[:])
            nc.gpsimd.memset(wdum[:, 3:4], 0.0)
            nc.vector.tensor_copy(wdum[:, 0:1], bfin[:])
            nc.vector.tensor_copy(wdum[:, 1:2], b1sb[:, 0, 0, :])
            nc.scalar.activation(wdum[:, 4:5], wdum[:, 3:4], Exp, bias=b1e[:, 0, 0, :], scale=1.0)
            nc.scalar.activation(wdum[:, 2:3], bfin[:], Identity, bias=bfin[:])

            # ---------------------------------------------------------------
            # Flat software pipeline over (tile, group).  Each group flows
            # d1(PE) -> Exp(ACT, per half) -> min(Pool, per half) ->
            # stt(DVE, per half) -> [pair done] d2(PE, deferred DLAG groups)
            # with the output stage of tile n interleaved into tile n+1's
            # group stream so no engine blocks at tile boundaries.
            # ---------------------------------------------------------------
            NF = n_tiles * NG
            DLAG = int(_os.environ.get("DLAG", "2"))  # pair-ready -> d2 emit lag
            state = {"xt": {}, "u": {}, "cw": {}, "o": {}, "pend": [], "outs": []}

            def emit_xt(f):
                n, g = divmod(f, NG)
                xt = xt_pool.tile([128, NB], F16, tag="xt")
                nc.sync.dma_start_transpose(
                    xt[:], x_d[NB * n:NB * (n + 1), 128 * g:128 * (g + 1)]
                )
                state["xt"][f] = xt

            def emit_d1(f):
                n, g = divmod(f, NG)
                if g == 0:
                    state["o"][n] = ps_o.tile([128, NB], F32, tag="o", name=f"o{n}")
                uqs = []
                xt = state["xt"].pop(f)
                for i in range(4):
                    uq = ps_u.tile([128, NB], F32, tag="u", name=f"u{f}_{i}")
                    nc.tensor.matmul(
                        uq[:], w1t[:, g, i, :], xt[:],
                        start=True, stop=True,
                    )
                    uqs.append(uq)
                state["u"][f] = uqs

            def emit_elem(f):
                n, g = divmod(f, NG)
                uqs = state["u"].pop(f)
                ew = ew_pool.tile([128, 4, NB], BF16, tag="E")
                tm = tm_pool.tile([128, 4, NB], BF16, tag="T")
                cw = cw_pool.tile([128, 4, NB], BF16, tag="C")
                d2src = []
                for i in range(4):
                    nc.scalar.activation(
                        ew[:, i, :], uqs[i][:], Exp,
                        bias=b1e[:, g, i, :], scale=1.0 / SCALE,
                    )
                    if i % 2 == 1:
                        sl = slice(i - 1, i + 1)
                        eng = nc.gpsimd if g < MIN_POOL_GROUPS else nc.vector
                        eng.tensor_scalar(
                            tm[:, sl, :], ew[:, sl, :], scalar1=SCALE,
                            scalar2=-SCALE, op0=Alu.min, op1=Alu.add,
                        )
                for i in range(4):
                    rt = 4 * g + i >= 4 * NG - RT_HALVES
                    if rt:
                        # two-stream form: R = ReLU(u65 + s*b1) on ACT;
                        # tm is streamed into dense2 directly (no stt)
                        nc.scalar.activation(
                            cw[:, i, :], uqs[i][:], Relu, bias=b1sb[:, g, i, :]
                        )
                        d2src += [(cw, i), (tm, i)]
                    else:
                        nc.vector.scalar_tensor_tensor(
                            cw[:, i, :], uqs[i][:], b1sb[:, g, i, :],
                            tm[:, i, :], op0=Alu.add, op1=Alu.max,
                        )
                        d2src.append((cw, i))
                state["cw"][f] = d2src
                state["pend"].append(f)       # group ready for deferred d2

            def emit_d2(fg):
                # 4-6 matmuls of group fg accumulate into partitions
                # 32p..32p+32 of the tile's shared o bank; the pair's chain
                # opens on the even group and closes on the odd one; all
                # tile_position (0, 32p) (chains must share one position).
                n, g = divmod(fg, NG)
                p = g // 2
                base = 32 * p
                o = state["o"][n]
                mms = state["cw"].pop(fg)
                for kseq, (st, i) in enumerate(mms):
                    nc.tensor.matmul(
                        o[base:base + 32, :],
                        w2t[:, g, i, :],
                        st[:, i, :],
                        start=(g % 2 == 0 and kseq == 0),
                        stop=(g % 2 == 1 and kseq == len(mms) - 1),
                        tile_position=(0, base),
                    )
                if g == NG - 1:
                    state["outs"].append(n)

            def emit_out_head(n):
                # bias add over all 128 q at once (ACT), fp16 out
                o = state["o"].pop(n)
                outq = outq_pool.tile([128, NB], F16, tag="outq")
                nc.scalar.activation(outq[:], o[:], Identity, bias=bfin[:])
                return outq

            def emit_out_tail(n, outq):
                # DMA xbar transpose [128q, 512b] -> 4x [128b, 128q], store
                ob = outb_pool.tile([128, 4, 128], F16, tag="outb")
                for k in range(4):
                    nc.sync.dma_start_transpose(
                        ob[:, k, :], outq[:, 128 * k:128 * (k + 1)]
                    )
                nc.sync.dma_start(
                    out_d[NB * n:NB * (n + 1), :].rearrange("(k p) q -> p k q", p=128),
                    ob[:],
                )

            loop_cm = tc.For_i(0, rep, 1) if rep > 1 else contextlib.nullcontext()
            with loop_cm:
                outq_pend = []
                for f in range(NF):
                    if f == 0:
                        for g in range(NG):
                            emit_xt(g)
                    pfl = int(_os.environ.get("PF_LEAD", "2"))
                    if (f + pfl) % NG == 0 and (f + pfl) < NF:
                        base_f = f + pfl
                        for g in range(NG):      # prefetch next tile's loads
                            emit_xt(base_f + g)
                    emit_d1(f)
                    if outq_pend:                # tout of tile n-1 between d1s
                        emit_out_tail(*outq_pend.pop(0))
                    while state["pend"] and state["pend"][0] <= f - DLAG:
                        fodd = state["pend"].pop(0)
                        emit_d2(fodd)
                        if state["outs"]:
                            n_done = state["outs"].pop(0)
                            outq_pend.append((n_done, emit_out_head(n_done)))
                    emit_elem(f)
                # drain
                while state["pend"]:
                    fodd = state["pend"].pop(0)
                    emit_d2(fodd)
                    if state["outs"]:
                        n_done = state["outs"].pop(0)
                        outq_pend.append((n_done, emit_out_head(n_done)))
                while outq_pend:
                    emit_out_tail(*outq_pend.pop(0))

    return nc


_CACHE = {}


def _get_nc(bc, has_b1, rep=1):
    key = (bc, has_b1, rep)
    if key not in _CACHE:
        _CACHE[key] = _build(bc, has_b1, rep)
    return _CACHE[key]


def kernel(x, W1, b1, gamma, beta, mov_mean, mov_var, W2, b2):
    x = np.asarray(x, np.float32).reshape(-1, C)
    B = x.shape[0]
    w1bd, w2t, bfin, b1sb, b1e, has_b1 = _host_pack(
        W1, b1, gamma, beta, mov_mean, mov_var, W2, b2
    )
    x8 = (x * XS).astype(np.float16)

    bc = B // NCORES
    nc = _get_nc(bc, has_b1)

    in_maps = [
        {
            "x": np.ascontiguousarray(x8[i * bc:(i + 1) * bc]),
            "w1bd": w1bd,
            "w2t": w2t,
            "bfin": bfin,
            "b1sb": b1sb,
            "b1e": b1e,
        }
        for i in range(NCORES)
    ]
    res = run_bass_kernel_spmd(nc, in_maps, list(range(NCORES)))
    kernel._last_results = res
    out = np.concatenate([res.results[i]["out"] for i in range(NCORES)], axis=0)
    return out.astype(np.float32)
